# revision 8
# baseline (speedup 1.0000x reference)
"""Expert-parallel MoE FFN for Trainium2 — one expert per NeuronCore (8 cores).

Strategy
--------
The reference computes, per token, the sum of top-2 expert FFN outputs (binary
combine mask, no gate weighting).  We shard along the expert axis: core ``e``
holds expert ``e``'s weights (W1[e], b1[e], W2[e], b2[e]) and processes only
the tokens that routed to it.

Host side (cheap, O(T*D*E) = 34 MFLOP):
  * gating softmax + top-2 (replicates jax.nn.softmax + jax.lax.top_k
    tie-breaking exactly: stable argsort on the fp32 scores, descending),
  * gather each expert's tokens, pad to a uniform capacity (all cores run the
    same NEFF), pre-transpose AND pre-pack every tensor into its exact SBUF
    layout ([128 partitions, flat free dim]) so each device DMA is a single
    trigger moving full-row (multi-KB) packets,
  * scatter-add the 8 per-expert outputs back into the [T, D] result.

Device side (the heavy part, ~18 GFLOP/core):
  hT = relu(W1^T-chained matmuls + b1);  yT = W2-chained matmuls + b2,
  everything kept in "transposed" layout: contraction dims live on SBUF
  partitions for both layers, so mm1's output feeds mm2 directly.
  bf16 inputs, fp32 PSUM accumulation.

Schedule (from perfetto traces): the tensor engine runs the 1536 matmuls
back-to-back at 1 col/cycle (bf16 peak); all the recoverable time is at the
edges.  Three measures tighten them:
  * a "boot" tensor fuses tile-0's first x half with W1's f=0 chunk so a
    single DMA trigger gates the first matmul,
  * W1 rides in f-ascending groups and W2 is packed m-major, so each weight
    group lands just ahead of the chain that consumes it (no mid-stream
    stalls waiting for the whole 8.4 MB of W2),
  * y returns as bf16 (halves the final, critical-path output transfer).
"""

import numpy as np
import ml_dtypes

import concourse.bacc as bacc
import concourse.mybir as mybir
import concourse.tile as tile
from concourse.bass_utils import run_bass_kernel_spmd
from concourse._compat import get_trn_type

D_MODEL = 1024
D_FF = 4096
N_EXP = 8
TOP_K = 2
KD = D_MODEL // 128  # 8 contraction chunks over d_model
KF = D_FF // 128  # 32 contraction chunks over d_ff

# W1 f-chunk groups (f=0 rides inside the boot tensor).  Sized so each group
# lands (at ~300+ GB/s aggregate) ahead of mm1's 1.23 us/chunk consumption.
W1F_GROUPS = [(1, 4), (4, 8), (8, 16), (16, 24), (24, 32)]
# W2 m-chunk groups, m-major packing: mm2's m-th chain only needs its group.
W2M_GROUPS = [(0, 4), (4, 8)]

BF16 = mybir.dt.bfloat16
F32 = mybir.dt.float32

_programs: dict[tuple, object] = {}


def _build_program(cap: int, tt: int):
    """Bass/Tile program: pre-packed [D,cap] tokens -> 2-layer FFN -> output."""
    assert cap % tt == 0
    nt = cap // tt
    ka = KD // 2
    nc = bacc.Bacc(get_trn_type() or "TRN2", target_bir_lowering=False, debug=False)

    # All inputs arrive pre-packed as [128, flat] in their SBUF layouts.
    # boot = [x tile-0 k-chunks 0..3 | W1 f-chunk 0 for all k]: one trigger
    # gates the first matmul chain.
    boot_cols = ka * tt + KD * 128
    boot_d = nc.dram_tensor("boot", [128, boot_cols], BF16, kind="ExternalInput").ap()
    x0b_d = nc.dram_tensor("x0b", [128, ka * tt], BF16, kind="ExternalInput").ap()
    if nt > 1:
        x1_d = nc.dram_tensor(
            "x1", [128, KD * (cap - tt)], BF16, kind="ExternalInput"
        ).ap()
    w1_d = [
        nc.dram_tensor(f"W1{g}", [128, KD * (fhi - flo) * 128], BF16,
                       kind="ExternalInput").ap()
        for g, (flo, fhi) in enumerate(W1F_GROUPS)
    ]
    w2_d = [
        nc.dram_tensor(f"W2{g}", [128, (mhi - mlo) * KF * 128], BF16,
                       kind="ExternalInput").ap()
        for g, (mlo, mhi) in enumerate(W2M_GROUPS)
    ]
    b1_d = nc.dram_tensor("b1", [128, KF], F32, kind="ExternalInput").ap()
    b2_d = nc.dram_tensor("b2", [128, KD], F32, kind="ExternalInput").ap()
    # One small DRAM tensor per (tile, m-chunk): a [128, tt] write with a
    # linear destination coalesces into a few big descriptors, while a
    # strided slice of one big tensor costs 128 tiny descriptors (~3 us on
    # the critical path for the final chunk).
    y_d = [
        [
            nc.dram_tensor(f"yT_{it}_{m}", [128, tt], BF16,
                           kind="ExternalOutput").ap()
            for m in range(KD)
        ]
        for it in range(nt)
    ]

    with tile.TileContext(nc) as tc:
        with (
            tc.tile_pool(name="sb", bufs=1) as sb,
            tc.tile_pool(name="hp", bufs=40) as hp,
            tc.tile_pool(name="yp", bufs=12) as yp,
            tc.tile_pool(name="pp1", bufs=6, space="PSUM") as pp1,
            tc.tile_pool(name="pp2", bufs=2, space="PSUM") as pp2,
        ):
            # ---- inputs --------------------------------------------------
            # All loads ride the single SWDGE ring serially, in exact
            # consumption order (one uncontended ring beats parallel rings —
            # concurrent HWDGE traffic starves the operand stream).
            boot_sb = sb.tile([128, boot_cols], BF16, tag="boot", name="boot_sb")
            x0b_sb = sb.tile([128, ka * tt], BF16, tag="x0b", name="x0b_sb")
            w1_tiles = [
            sb.tile([128, KD * (fhi - flo) * 128], BF16, tag=f"w1g{g}",
                        name=f"w1g{g}")
                for g, (flo, fhi) in enumerate(W1F_GROUPS)
            ]
            b1_sb = sb.tile([128, KF], F32, tag="b1", name="b1_sb")
            b2_sb = sb.tile([128, KD], F32, tag="b2", name="b2_sb")
            w2_tiles = [
                sb.tile([128, (mhi - mlo) * KF * 128], BF16, tag=f"w2g{g}",
                        name=f"w2g{g}")
                for g, (mlo, mhi) in enumerate(W2M_GROUPS)
            ]
            if nt > 1:
                x1_sb = sb.tile([128, KD * (cap - tt)], BF16, tag="x1", name="x1_sb")

            # Warm-up spinner: ~16 dependency-free matmuls on a memset tile
            # run while the boot DMA is in flight, spinning the PE out of its
            # idle clock state so the real stream starts at full rate.
            zero_sb = sb.tile([128, 128], BF16, tag="zero", name="zero_sb")
            nc.gpsimd.memset(zero_sb[:], 0)
            for i in range(16):
                # tag "ps2": share the mm2 pool's two PSUM banks (a tag of
                # its own would allocate two more banks than exist).
                wp = pp2.tile([128, 128], F32, tag="ps2", name=f"warm{i}")
                nc.tensor.matmul(wp[:], zero_sb[:], zero_sb[:], start=True,
                                 stop=True)

            # All input loads ride the sync queue serially, in exact
            # consumption order: one uncontended stream (sharded over all 16
            # HW queues) keeps the boot tensor's landing — which gates the
            # first matmul — as early as possible.
            nc.sync.dma_start(boot_sb[:], boot_d)
            nc.sync.dma_start(x0b_sb[:], x0b_d)
            nc.sync.dma_start(w1_tiles[0][:], w1_d[0])
            nc.sync.dma_start(b1_sb[:], b1_d)
            nc.sync.dma_start(b2_sb[:], b2_d)
            for g in range(1, len(W1F_GROUPS)):
                nc.sync.dma_start(w1_tiles[g][:], w1_d[g])
            for g in range(len(W2M_GROUPS)):
                nc.sync.dma_start(w2_tiles[g][:], w2_d[g])
            if nt > 1:
                nc.sync.dma_start(x1_sb[:], x1_d)

            def x_rhs(k, it):
                if it == 0:
                    if k < ka:
                        return boot_sb[:, k * tt : (k + 1) * tt]
                    return x0b_sb[:, (k - ka) * tt : (k - ka + 1) * tt]
                rest = cap - tt
                lo = k * rest + (it - 1) * tt
                return x1_sb[:, lo : lo + tt]

            def w1_lhsT(k, f):
                if f == 0:
                    base = ka * tt + k * 128
                    return boot_sb[:, base : base + 128]
                for (flo, fhi), t in zip(W1F_GROUPS, w1_tiles):
                    if flo <= f < fhi:
                        base = k * (fhi - flo) * 128 + (f - flo) * 128
                        return t[:, base : base + 128]
                raise AssertionError

            def w2_lhsT(f, m):
                for (mlo, mhi), t in zip(W2M_GROUPS, w2_tiles):
                    if mlo <= m < mhi:
                        base = (m - mlo) * KF * 128 + f * 128
                        return t[:, base : base + 128]
                raise AssertionError

            # ---- compute --------------------------------------------------
            for it in range(nt):
                # mm1: hT[f*128+p, t] = relu(sum_d W1[d, f*128+p]*xT[d, t] + b1)
                h_tiles = []
                for f in range(KF):
                    ps = pp1.tile([128, tt], F32, tag="ps1", name=f"ps1_{it}_{f}")
                    for k in range(KD):
                        nc.tensor.matmul(
                            ps[:],
                            w1_lhsT(k, f),
                            x_rhs(k, it),
                            start=(k == 0),
                            stop=(k == KD - 1),
                        )
                    ht = hp.tile([128, tt], BF16, tag="h", name=f"h_{it}_{f}")
                    nc.scalar.activation(
                        ht[:],
                        ps[:],
                        mybir.ActivationFunctionType.Relu,
                        bias=b1_sb[:, f : f + 1],
                    )
                    h_tiles.append(ht)

                # mm2: yT[m*128+p, t] = sum_f W2[f, m*128+p] * hT[f, t] + b2
                for m in range(KD):
                    ps2 = pp2.tile([128, tt], F32, tag="ps2", name=f"ps2_{it}_{m}")
                    for f in range(KF):
                        nc.tensor.matmul(
                            ps2[:],
                            w2_lhsT(f, m),
                            h_tiles[f][:],
                            start=(f == 0),
                            stop=(f == KF - 1),
                        )
                    yt = yp.tile([128, tt], BF16, tag="y", name=f"y_{it}_{m}")
                    nc.vector.tensor_scalar_add(yt[:], ps2[:], b2_sb[:, m : m + 1])
                    nc.sync.dma_start(y_d[it][m], yt[:])

    nc.compile()
    return nc


def _gating_topk(x, Wg, bg):
    """Replicates jax.nn.softmax + jax.lax.top_k(..., 2) in fp32 numpy."""
    logits = x @ Wg + bg
    m = logits.max(axis=1, keepdims=True)
    e = np.exp(logits - m)
    scores = e / e.sum(axis=1, keepdims=True)
    # top_k: descending, ties broken toward the lower index (stable).
    order = np.argsort(-scores, axis=1, kind="stable")
    return order[:, :TOP_K]


def _capacity(max_count):
    # Token tile <= 384: keeps one fp32 PSUM bank per matmul (<=512) AND the
    # resident-weights SBUF budget valid for capacities well beyond the
    # ~1024+-67 expert loads this distribution produces.
    nt = max(1, -(-max_count // 384))
    tt = -(-max_count // nt)
    tt = -(-tt // 4) * 4  # multiple of 4 for aligned fp32 rows
    return nt * tt, tt


def _pack_k128(a):
    """[K*128, F] -> [128, K*F]: partition-major packing of the SBUF layout."""
    k128, f = a.shape
    return np.ascontiguousarray(
        a.reshape(k128 // 128, 128, f).transpose(1, 0, 2).reshape(128, -1)
    )


def _prepare(x, Wg, bg, W1, b1, W2, b2):
    x = np.ascontiguousarray(np.asarray(x, dtype=np.float32))
    topk = _gating_topk(x, np.asarray(Wg, np.float32), np.asarray(bg, np.float32))
    idx = [np.nonzero((topk == e).any(axis=1))[0] for e in range(N_EXP)]
    counts = [len(i) for i in idx]
    cap, tt = _capacity(max(counts))
    nt = cap // tt
    ka = KD // 2

    bf16 = ml_dtypes.bfloat16
    in_maps = []
    for e in range(N_EXP):
        xg = np.zeros((cap, D_MODEL), np.float32)
        xg[: counts[e]] = x[idx[e]]
        xT = np.ascontiguousarray(xg.T).astype(bf16)  # [D, cap]
        xTp = _pack_k128(xT).reshape(128, KD, cap)  # [128, k, c]
        w1 = np.asarray(W1[e], np.float32).astype(bf16)  # [D, DFF]
        w1p = _pack_k128(w1).reshape(128, KD, D_FF)  # [128, k, f]
        w2 = np.asarray(W2[e], np.float32).astype(bf16)  # [DFF, D]
        w2p = _pack_k128(w2).reshape(128, KF, D_MODEL)  # [128, f, m]
        w2m = np.ascontiguousarray(
            w2p.reshape(128, KF, KD, 128).transpose(0, 2, 1, 3)
        )  # [128, m, f, 128] — m-major
        m = {
            "boot": np.ascontiguousarray(
                np.concatenate(
                    [
                        xTp[:, :ka, :tt].reshape(128, -1),
                        w1p[:, :, :128].reshape(128, -1),
                    ],
                    axis=1,
                )
            ),
            "x0b": np.ascontiguousarray(xTp[:, ka:, :tt]).reshape(128, -1),
            "b1": np.ascontiguousarray(
                np.asarray(b1[e], np.float32).reshape(KF, 128).T
            ),
            "b2": np.ascontiguousarray(
                np.asarray(b2[e], np.float32).reshape(KD, 128).T
            ),
        }
        if nt > 1:
            m["x1"] = np.ascontiguousarray(xTp[:, :, tt:]).reshape(128, -1)
        for g, (flo, fhi) in enumerate(W1F_GROUPS):
            m[f"W1{g}"] = np.ascontiguousarray(
                w1p[:, :, flo * 128 : fhi * 128]
            ).reshape(128, -1)
        for g, (mlo, mhi) in enumerate(W2M_GROUPS):
            m[f"W2{g}"] = np.ascontiguousarray(w2m[:, mlo:mhi]).reshape(128, -1)
        in_maps.append(m)
    return x, idx, counts, cap, tt, in_maps


def _run(x, Wg, bg, W1, b1, W2, b2, **run_kwargs):
    x, idx, counts, cap, tt, in_maps = _prepare(x, Wg, bg, W1, b1, W2, b2)
    key = (cap, tt)
    prog = _programs.get(key)
    if prog is None:
        prog = _programs.setdefault(key, _build_program(cap, tt))
    res = run_bass_kernel_spmd(
        prog, in_maps, core_ids=list(range(N_EXP)), **run_kwargs
    )
    nt = cap // tt
    out = np.zeros_like(x)
    yT = np.empty((D_MODEL, cap), np.float32)
    for e in range(N_EXP):
        for it in range(nt):
            for m in range(KD):
                yT[m * 128 : (m + 1) * 128, it * tt : (it + 1) * tt] = np.asarray(
                    res.results[e][f"yT_{it}_{m}"], np.float32
                )
        out[idx[e]] += yT[:, : counts[e]].T
    return out, res


def kernel(x, Wg, bg, W1, b1, W2, b2):
    out, _ = _run(x, Wg, bg, W1, b1, W2, b2)
    return out


# revision 10
# speedup vs baseline: 1.0073x; 1.0073x over previous
"""Expert-parallel MoE FFN for Trainium2 — one expert per NeuronCore (8 cores).

Strategy
--------
The reference computes, per token, the sum of top-2 expert FFN outputs (binary
combine mask, no gate weighting).  We shard along the expert axis: core ``e``
holds expert ``e``'s weights (W1[e], b1[e], W2[e], b2[e]) and processes only
the tokens that routed to it.

Host side (cheap, O(T*D*E) = 34 MFLOP):
  * gating softmax + top-2 (replicates jax.nn.softmax + jax.lax.top_k
    tie-breaking exactly: stable argsort on the fp32 scores, descending),
  * gather each expert's tokens, pad to a uniform capacity (all cores run the
    same NEFF), pre-transpose AND pre-pack every tensor into its exact SBUF
    layout ([128 partitions, flat free dim]) so each device DMA is a single
    trigger moving full-row (multi-KB) packets,
  * scatter-add the 8 per-expert outputs back into the [T, D] result.

Device side (the heavy part, ~18 GFLOP/core):
  hT = relu(W1^T-chained matmuls + b1);  yT = W2-chained matmuls + b2,
  everything kept in "transposed" layout: contraction dims live on SBUF
  partitions for both layers, so mm1's output feeds mm2 directly.
  bf16 inputs, fp32 PSUM accumulation.

Schedule (from perfetto traces): the tensor engine runs the 1536 matmuls
back-to-back at 1 col/cycle (bf16 peak); all the recoverable time is at the
edges.  Three measures tighten them:
  * a "boot" tensor fuses tile-0's first x half with W1's f=0 chunk so a
    single DMA trigger gates the first matmul,
  * W1 rides in f-ascending groups and W2 is packed m-major, so each weight
    group lands just ahead of the chain that consumes it (no mid-stream
    stalls waiting for the whole 8.4 MB of W2),
  * y returns as bf16 (halves the final, critical-path output transfer).
"""

import numpy as np
import ml_dtypes

import concourse.bacc as bacc
import concourse.mybir as mybir
import concourse.tile as tile
from concourse.bass_utils import run_bass_kernel_spmd
from concourse._compat import get_trn_type

D_MODEL = 1024
D_FF = 4096
N_EXP = 8
TOP_K = 2
KD = D_MODEL // 128  # 8 contraction chunks over d_model
KF = D_FF // 128  # 32 contraction chunks over d_ff

# W1 f-chunk groups (f=0 rides inside the boot tensor).  Sized so each group
# lands (at ~300+ GB/s aggregate) ahead of mm1's 1.23 us/chunk consumption.
W1F_GROUPS = [(1, 4), (4, 8), (8, 16), (16, 24), (24, 32)]
# W2 m-chunk groups, m-major packing: mm2's m-th chain only needs its group.
W2M_GROUPS = [(0, 4), (4, 8)]

BF16 = mybir.dt.bfloat16
F32 = mybir.dt.float32

_programs: dict[tuple, object] = {}


def _build_program(cap: int, tt: int):
    """Bass/Tile program: pre-packed [D,cap] tokens -> 2-layer FFN -> output."""
    assert cap % tt == 0
    nt = cap // tt
    ka = KD // 2
    nc = bacc.Bacc(get_trn_type() or "TRN2", target_bir_lowering=False, debug=False)

    # All inputs arrive pre-packed as [128, flat] in their SBUF layouts.
    # boot = [x tile-0 k-chunks 0..3 | W1 f-chunk 0 for all k]: one trigger
    # gates the first matmul chain.
    boot_cols = ka * tt + KD * 128
    boot_d = nc.dram_tensor("boot", [128, boot_cols], BF16, kind="ExternalInput").ap()
    x0b_d = nc.dram_tensor("x0b", [128, ka * tt], BF16, kind="ExternalInput").ap()
    if nt > 1:
        x1_d = nc.dram_tensor(
            "x1", [128, KD * (cap - tt)], BF16, kind="ExternalInput"
        ).ap()
    w1_d = [
        nc.dram_tensor(f"W1{g}", [128, KD * (fhi - flo) * 128], BF16,
                       kind="ExternalInput").ap()
        for g, (flo, fhi) in enumerate(W1F_GROUPS)
    ]
    w2_d = [
        nc.dram_tensor(f"W2{g}", [128, (mhi - mlo) * KF * 128], BF16,
                       kind="ExternalInput").ap()
        for g, (mlo, mhi) in enumerate(W2M_GROUPS)
    ]
    b1_d = nc.dram_tensor("b1", [128, KF], F32, kind="ExternalInput").ap()
    b2_d = nc.dram_tensor("b2", [128, KD], F32, kind="ExternalInput").ap()
    # One small DRAM tensor per (tile, m-chunk): a [128, tt] write with a
    # linear destination coalesces into a few big descriptors, while a
    # strided slice of one big tensor costs 128 tiny descriptors (~3 us on
    # the critical path for the final chunk).
    y_d = [
        [
            nc.dram_tensor(f"yT_{it}_{m}", [128, tt], BF16,
                           kind="ExternalOutput").ap()
            for m in range(KD)
        ]
        for it in range(nt)
    ]

    with tile.TileContext(nc) as tc:
        with (
            tc.tile_pool(name="sb", bufs=1) as sb,
            tc.tile_pool(name="hp", bufs=40) as hp,
            tc.tile_pool(name="yp", bufs=12) as yp,
            tc.tile_pool(name="pp1", bufs=6, space="PSUM") as pp1,
            tc.tile_pool(name="pp2", bufs=2, space="PSUM") as pp2,
        ):
            # ---- inputs --------------------------------------------------
            # All loads ride the single SWDGE ring serially, in exact
            # consumption order (one uncontended ring beats parallel rings —
            # concurrent HWDGE traffic starves the operand stream).
            boot_sb = sb.tile([128, boot_cols], BF16, tag="boot", name="boot_sb")
            x0b_sb = sb.tile([128, ka * tt], BF16, tag="x0b", name="x0b_sb")
            w1_tiles = [
            sb.tile([128, KD * (fhi - flo) * 128], BF16, tag=f"w1g{g}",
                        name=f"w1g{g}")
                for g, (flo, fhi) in enumerate(W1F_GROUPS)
            ]
            b1_sb = sb.tile([128, KF], F32, tag="b1", name="b1_sb")
            b2_sb = sb.tile([128, KD], F32, tag="b2", name="b2_sb")
            w2_tiles = [
                sb.tile([128, (mhi - mlo) * KF * 128], BF16, tag=f"w2g{g}",
                        name=f"w2g{g}")
                for g, (mlo, mhi) in enumerate(W2M_GROUPS)
            ]
            if nt > 1:
                x1_sb = sb.tile([128, KD * (cap - tt)], BF16, tag="x1", name="x1_sb")

            # Warm-up spinner: dependency-free matmuls run back-to-back while
            # the boot DMA is in flight.  The PE ramps to full rate only
            # after ~8 us of *sustained* load (measured: the first real
            # matmuls run at 1.4-3x duration otherwise), so keep it busy
            # from sequencer-init until just before the boot tensor lands.
            zero_sb = sb.tile([128, 364], BF16, tag="zero", name="zero_sb")
            nc.gpsimd.memset(zero_sb[:], 0)
            for i in range(30):
                # tag "ps2": share the mm2 pool's two PSUM banks (a tag of
                # its own would allocate two more banks than exist).
                wp = pp2.tile([128, 364], F32, tag="ps2", name=f"warm{i}")
                nc.tensor.matmul(wp[:], zero_sb[:, :128], zero_sb[:],
                                 start=True, stop=True)

            # All input loads ride the gpsimd queue serially, in exact
            # consumption order (sharded over all 16 HW queues at full ring
            # bandwidth).  w1f1-3 is ordered before x0b: the first chain
            # only reaches x0b's k-half ~0.6 us in, while f=1 follows f=0
            # ~1.2 us in but needs the whole group landed.
            nc.gpsimd.dma_start(boot_sb[:], boot_d)
            nc.gpsimd.dma_start(w1_tiles[0][:], w1_d[0])
            nc.gpsimd.dma_start(x0b_sb[:], x0b_d)
            nc.gpsimd.dma_start(b1_sb[:], b1_d)
            nc.gpsimd.dma_start(b2_sb[:], b2_d)
            for g in range(1, len(W1F_GROUPS)):
                nc.gpsimd.dma_start(w1_tiles[g][:], w1_d[g])
            for g in range(len(W2M_GROUPS)):
                nc.gpsimd.dma_start(w2_tiles[g][:], w2_d[g])
            if nt > 1:
                nc.gpsimd.dma_start(x1_sb[:], x1_d)

            def x_rhs(k, it):
                if it == 0:
                    if k < ka:
                        return boot_sb[:, k * tt : (k + 1) * tt]
                    return x0b_sb[:, (k - ka) * tt : (k - ka + 1) * tt]
                rest = cap - tt
                lo = k * rest + (it - 1) * tt
                return x1_sb[:, lo : lo + tt]

            def w1_lhsT(k, f):
                if f == 0:
                    base = ka * tt + k * 128
                    return boot_sb[:, base : base + 128]
                for (flo, fhi), t in zip(W1F_GROUPS, w1_tiles):
                    if flo <= f < fhi:
                        base = k * (fhi - flo) * 128 + (f - flo) * 128
                        return t[:, base : base + 128]
                raise AssertionError

            def w2_lhsT(f, m):
                for (mlo, mhi), t in zip(W2M_GROUPS, w2_tiles):
                    if mlo <= m < mhi:
                        base = (m - mlo) * KF * 128 + f * 128
                        return t[:, base : base + 128]
                raise AssertionError

            # ---- compute --------------------------------------------------
            for it in range(nt):
                # mm1: hT[f*128+p, t] = relu(sum_d W1[d, f*128+p]*xT[d, t] + b1)
                h_tiles = []
                for f in range(KF):
                    ps = pp1.tile([128, tt], F32, tag="ps1", name=f"ps1_{it}_{f}")
                    for k in range(KD):
                        nc.tensor.matmul(
                            ps[:],
                            w1_lhsT(k, f),
                            x_rhs(k, it),
                            start=(k == 0),
                            stop=(k == KD - 1),
                        )
                    ht = hp.tile([128, tt], BF16, tag="h", name=f"h_{it}_{f}")
                    nc.scalar.activation(
                        ht[:],
                        ps[:],
                        mybir.ActivationFunctionType.Relu,
                        bias=b1_sb[:, f : f + 1],
                    )
                    h_tiles.append(ht)

                # mm2: yT[m*128+p, t] = sum_f W2[f, m*128+p] * hT[f, t] + b2
                for m in range(KD):
                    ps2 = pp2.tile([128, tt], F32, tag="ps2", name=f"ps2_{it}_{m}")
                    for f in range(KF):
                        nc.tensor.matmul(
                            ps2[:],
                            w2_lhsT(f, m),
                            h_tiles[f][:],
                            start=(f == 0),
                            stop=(f == KF - 1),
                        )
                    yt = yp.tile([128, tt], BF16, tag="y", name=f"y_{it}_{m}")
                    nc.vector.tensor_scalar_add(yt[:], ps2[:], b2_sb[:, m : m + 1])
                    if it == nt - 1 and m == KD - 1:
                        # Final chunk is on the critical path and its DMA is
                        # descriptor-latency-bound (one descriptor per
                        # partition): split across two queue engines.
                        nc.sync.dma_start(y_d[it][m][0:64, :], yt[0:64, :])
                        nc.gpsimd.dma_start(y_d[it][m][64:128, :], yt[64:128, :])
                    else:
                        nc.sync.dma_start(y_d[it][m], yt[:])

    nc.compile()
    return nc


def _gating_topk(x, Wg, bg):
    """Replicates jax.nn.softmax + jax.lax.top_k(..., 2) in fp32 numpy."""
    logits = x @ Wg + bg
    m = logits.max(axis=1, keepdims=True)
    e = np.exp(logits - m)
    scores = e / e.sum(axis=1, keepdims=True)
    # top_k: descending, ties broken toward the lower index (stable).
    order = np.argsort(-scores, axis=1, kind="stable")
    return order[:, :TOP_K]


def _capacity(max_count):
    # Token tile <= 384: keeps one fp32 PSUM bank per matmul (<=512) AND the
    # resident-weights SBUF budget valid for capacities well beyond the
    # ~1024+-67 expert loads this distribution produces.
    nt = max(1, -(-max_count // 384))
    tt = -(-max_count // nt)
    tt = -(-tt // 4) * 4  # multiple of 4 for aligned fp32 rows
    return nt * tt, tt


def _pack_k128(a):
    """[K*128, F] -> [128, K*F]: partition-major packing of the SBUF layout."""
    k128, f = a.shape
    return np.ascontiguousarray(
        a.reshape(k128 // 128, 128, f).transpose(1, 0, 2).reshape(128, -1)
    )


def _prepare(x, Wg, bg, W1, b1, W2, b2):
    x = np.ascontiguousarray(np.asarray(x, dtype=np.float32))
    topk = _gating_topk(x, np.asarray(Wg, np.float32), np.asarray(bg, np.float32))
    idx = [np.nonzero((topk == e).any(axis=1))[0] for e in range(N_EXP)]
    counts = [len(i) for i in idx]
    cap, tt = _capacity(max(counts))
    nt = cap // tt
    ka = KD // 2

    bf16 = ml_dtypes.bfloat16
    in_maps = []
    for e in range(N_EXP):
        xg = np.zeros((cap, D_MODEL), np.float32)
        xg[: counts[e]] = x[idx[e]]
        xT = np.ascontiguousarray(xg.T).astype(bf16)  # [D, cap]
        xTp = _pack_k128(xT).reshape(128, KD, cap)  # [128, k, c]
        w1 = np.asarray(W1[e], np.float32).astype(bf16)  # [D, DFF]
        w1p = _pack_k128(w1).reshape(128, KD, D_FF)  # [128, k, f]
        w2 = np.asarray(W2[e], np.float32).astype(bf16)  # [DFF, D]
        w2p = _pack_k128(w2).reshape(128, KF, D_MODEL)  # [128, f, m]
        w2m = np.ascontiguousarray(
            w2p.reshape(128, KF, KD, 128).transpose(0, 2, 1, 3)
        )  # [128, m, f, 128] — m-major
        m = {
            "boot": np.ascontiguousarray(
                np.concatenate(
                    [
                        xTp[:, :ka, :tt].reshape(128, -1),
                        w1p[:, :, :128].reshape(128, -1),
                    ],
                    axis=1,
                )
            ),
            "x0b": np.ascontiguousarray(xTp[:, ka:, :tt]).reshape(128, -1),
            "b1": np.ascontiguousarray(
                np.asarray(b1[e], np.float32).reshape(KF, 128).T
            ),
            "b2": np.ascontiguousarray(
                np.asarray(b2[e], np.float32).reshape(KD, 128).T
            ),
        }
        if nt > 1:
            m["x1"] = np.ascontiguousarray(xTp[:, :, tt:]).reshape(128, -1)
        for g, (flo, fhi) in enumerate(W1F_GROUPS):
            m[f"W1{g}"] = np.ascontiguousarray(
                w1p[:, :, flo * 128 : fhi * 128]
            ).reshape(128, -1)
        for g, (mlo, mhi) in enumerate(W2M_GROUPS):
            m[f"W2{g}"] = np.ascontiguousarray(w2m[:, mlo:mhi]).reshape(128, -1)
        in_maps.append(m)
    return x, idx, counts, cap, tt, in_maps


def _run(x, Wg, bg, W1, b1, W2, b2, **run_kwargs):
    x, idx, counts, cap, tt, in_maps = _prepare(x, Wg, bg, W1, b1, W2, b2)
    key = (cap, tt)
    prog = _programs.get(key)
    if prog is None:
        prog = _programs.setdefault(key, _build_program(cap, tt))
    res = run_bass_kernel_spmd(
        prog, in_maps, core_ids=list(range(N_EXP)), **run_kwargs
    )
    nt = cap // tt
    out = np.zeros_like(x)
    yT = np.empty((D_MODEL, cap), np.float32)
    for e in range(N_EXP):
        for it in range(nt):
            for m in range(KD):
                yT[m * 128 : (m + 1) * 128, it * tt : (it + 1) * tt] = np.asarray(
                    res.results[e][f"yT_{it}_{m}"], np.float32
                )
        out[idx[e]] += yT[:, : counts[e]].T
    return out, res


def kernel(x, Wg, bg, W1, b1, W2, b2):
    out, _ = _run(x, Wg, bg, W1, b1, W2, b2)
    return out
